# revision 7
# baseline (speedup 1.0000x reference)
"""DeepSeek-V3 MoE block on 8 trn2 NeuronCores.

Expert-parallel sparse MoE:
  - host computes routing indices (dispatch metadata) in fp32 numpy
  - experts sorted by token count into 4 tiers of 8; core c slot k holds the
    (8k+c)-th busiest expert; slot capacity = padded max count of its tier so
    every core runs the identical program (SPMD) with zero weight duplication
  - device computes: gate logits (true fp32 matmul), sigmoid+grouped-top-k
    routing weights, per-expert gated MLP (fp32r matmuls), shared-expert MLP
    (intermediate dim sharded 8-way), combine-scaling at the down projection
  - host sums the 8 shared-expert partials and scatter-adds the routed rows
"""

import os
import sys

sys.path.insert(0, "/opt/trn_rl_repo")

import numpy as np

import concourse.bacc as bacc
import concourse.bass as bass
import concourse.mybir as mybir
import concourse.tile as tile
from concourse.bass_utils import run_bass_kernel_spmd

F32 = mybir.dt.float32
F32R = mybir.dt.float32r
AF = mybir.ActivationFunctionType
ALU = mybir.AluOpType
AX = mybir.AxisListType

T, H, I, IS, E = 1024, 1024, 512, 2048, 32
G, TOPK_GROUP, TOP_K = 8, 4, 8
SCALE = 2.5
NCORES = 8
S = E // NCORES          # expert slots per core
ISH = IS // NCORES       # shared-expert intermediate shard
P128 = 128

LAST_RESULTS = None      # BassKernelResults of the most recent run


def _install_ntff_hook():
    """Provide antenv.axon_hooks + the ctypes NTFF profile hook when the
    container image lacks them (needed only for trace=True)."""
    import contextlib
    import ctypes
    import types

    try:
        from antenv.axon_hooks import get_axon_ntff_profile_hook  # noqa: F401
        return True
    except ImportError:
        pass
    try:
        import antenv
        so_path = "/opt/axon/libaxon_pjrt.so"
        lib = ctypes.CDLL(so_path)
        if not hasattr(lib, "axon_start_nrt_profile"):
            return False
        lib.axon_start_nrt_profile.argtypes = [
            ctypes.POINTER(ctypes.c_int64), ctypes.c_size_t]
        lib.axon_start_nrt_profile.restype = ctypes.c_int64
        lib.axon_stop_nrt_profile.argtypes = [ctypes.c_char_p]
        lib.axon_stop_nrt_profile.restype = ctypes.c_int64

        @contextlib.contextmanager
        def _hook(output_dir, device_ids):
            import jax
            jax.devices()
            if device_ids:
                ids = (ctypes.c_int64 * len(device_ids))(*device_ids)
                rc = lib.axon_start_nrt_profile(ids, len(device_ids))
            else:
                rc = lib.axon_start_nrt_profile(None, 0)
            if rc != 0:
                raise RuntimeError(f"axon_start_nrt_profile rc={rc}")
            try:
                yield
            finally:
                n = lib.axon_stop_nrt_profile(str(output_dir).encode())
                print(f"ntff profile: {n} file(s) -> {output_dir}",
                      file=sys.stderr)

        mod = types.ModuleType("antenv.axon_hooks")
        _state = {"hook": _hook}
        mod.set_axon_ntff_profile_hook = lambda h: _state.__setitem__("hook", h)
        mod.get_axon_ntff_profile_hook = lambda: _state["hook"]
        sys.modules["antenv.axon_hooks"] = mod
        antenv.axon_hooks = mod
        return True
    except Exception:
        return False


def _host_routing(x, gate_w, e_bias):
    """fp32 numpy mirror of reference._routing_combine; returns emask [T,E]."""
    logits = x.astype(np.float32) @ gate_w.T.astype(np.float32)
    scores = 1.0 / (1.0 + np.exp(-logits))
    swb = scores + e_bias[None, :]
    swb_g = swb.reshape(T, G, E // G)
    gs = np.sort(swb_g, axis=-1)[..., -2:].sum(-1)          # top-2 sum per group
    thr4 = np.sort(gs, axis=-1)[:, -TOPK_GROUP][:, None]
    gmask = (gs >= thr4).astype(np.float32)
    smask = np.repeat(gmask, E // G, axis=-1)
    masked = swb * smask
    thr8 = np.sort(masked, axis=-1)[:, -TOP_K][:, None]
    emask = masked >= thr8
    return emask


def _pad128(n):
    return max(P128, ((int(n) + P128 - 1) // P128) * P128)


def _chunks(p, limit=512):
    """Split width p into chunks <= limit (each a multiple of 128)."""
    out = []
    o = 0
    while o < p:
        w = min(limit, p - o)
        out.append((o, w))
        o += w
    return out


def _build_program(P):
    """Emit the SPMD Bass program for slot capacities P (list of S ints)."""
    nc = bacc.Bacc(target_bir_lowering=False, debug=False)
    max_nc = P[0] // P128

    # ---- DRAM parameters (per-core data arrives via in_maps) ----
    xe_d = [nc.dram_tensor(f"xe{k}", [H, P[k]], F32R, kind="ExternalInput")
            for k in range(S)]
    wg_d = [nc.dram_tensor(f"wg{k}", [H, I], F32R, kind="ExternalInput")
            for k in range(S)]
    wu_d = [nc.dram_tensor(f"wu{k}", [H, I], F32R, kind="ExternalInput")
            for k in range(S)]
    wd_d = [nc.dram_tensor(f"wd{k}", [I, H], F32R, kind="ExternalInput")
            for k in range(S)]
    oh_d = [nc.dram_tensor(f"oh{k}", [P128, (P[k] // P128) * E], F32,
                           kind="ExternalInput") for k in range(S)]
    em_d = [nc.dram_tensor(f"em{k}", [P128, (P[k] // P128) * E], F32,
                           kind="ExternalInput") for k in range(S)]
    gwt_d = nc.dram_tensor("gwt", [P128, H // P128, E], F32R, kind="ExternalInput")
    xt_d = nc.dram_tensor("xt", [H, T], F32R, kind="ExternalInput")
    wsg_d = nc.dram_tensor("wsg", [H, ISH], F32R, kind="ExternalInput")
    wsu_d = nc.dram_tensor("wsu", [H, ISH], F32R, kind="ExternalInput")
    wsd_d = nc.dram_tensor("wsd", [ISH, H], F32R, kind="ExternalInput")
    ro_d = [nc.dram_tensor(f"ro{k}", [P[k], H], F32, kind="ExternalOutput")
            for k in range(S)]
    so_d = nc.dram_tensor("so", [T, H], F32, kind="ExternalOutput")

    HT = H // P128  # 8 h-tiles

    with tile.TileContext(nc) as tc:
        with (
            tc.tile_pool(name="const", bufs=1) as cpool,
            tc.tile_pool(name="xe", bufs=8) as xepool,
            tc.tile_pool(name="w", bufs=10) as wpool,
            tc.tile_pool(name="wd", bufs=5) as wdpool,
            tc.tile_pool(name="acts", bufs=5) as apool,
            tc.tile_pool(name="rt", bufs=1) as rpool,
            tc.tile_pool(name="scl", bufs=1) as spool,
            tc.tile_pool(name="stage", bufs=4) as stpool,
            tc.tile_pool(name="ps", bufs=2, space="PSUM") as ps,
        ):
            # ---- constants ----
            gwt = cpool.tile([P128, HT, E], F32R, tag="gwt")
            nc.sync.dma_start(out=gwt[:], in_=gwt_d[:])
            oh_sb, em_sb = [], []
            for k in range(S):
                t = cpool.tile([P128, (P[k] // P128) * E], F32, tag=f"oh{k}")
                nc.sync.dma_start(out=t[:], in_=oh_d[k][:])
                oh_sb.append(t)
                t2 = cpool.tile([P128, (P[k] // P128) * E], F32, tag=f"em{k}")
                nc.sync.dma_start(out=t2[:], in_=em_d[k][:])
                em_sb.append(t2)

            # ---- per-slot input DMA + routing ----
            xe_sb = []     # xe_sb[k][h] : [128, P[k]]
            scale_sb = []  # scale_sb[k] : [128, n_c]  (combine weight per token)
            for k in range(S):
                n_c = P[k] // P128
                xs = []
                for h in range(HT):
                    xt_t = xepool.tile([P128, P[k]], F32R, tag=f"xe{k}")
                    nc.sync.dma_start(out=xt_t[:],
                                      in_=xe_d[k][h * P128:(h + 1) * P128, :])
                    xs.append(xt_t)
                xe_sb.append(xs)

                # logits for this slot's gathered tokens: true fp32 matmul
                lg = ps.tile([P128, n_c * E], F32, tag="lg")
                for cc in range(n_c):
                    for h in range(HT):
                        nc.tensor.matmul(
                            lg[:, cc * E:(cc + 1) * E],
                            lhsT=xs[h][:, cc * P128:(cc + 1) * P128],
                            rhs=gwt[:, h, :],
                            start=(h == 0), stop=(h == HT - 1),
                        )

                # combine weights: sigmoid scores; selection mask comes
                # from the host dispatch (bit-identical to reference topk)
                scores = rpool.tile([P128, n_c * E], F32, tag="scores")
                nc.scalar.activation(scores[:], lg[:], AF.Sigmoid)
                sct = rpool.tile([P128, n_c * E], F32, tag="sct")
                nc.vector.tensor_mul(sct[:], scores[:], em_sb[k][:])
                den = rpool.tile([P128, n_c], F32, tag="den")
                nc.vector.tensor_reduce(
                    den[:], sct[:].rearrange("p (c e) -> p c e", e=E),
                    axis=AX.X, op=ALU.add)
                num_t = rpool.tile([P128, n_c * E], F32, tag="numt")
                nc.vector.tensor_mul(num_t[:], scores[:], oh_sb[k][:])
                num = rpool.tile([P128, n_c], F32, tag="num")
                nc.vector.tensor_reduce(
                    num[:], num_t[:].rearrange("p (c e) -> p c e", e=E),
                    axis=AX.X, op=ALU.add)
                rec = rpool.tile([P128, n_c], F32, tag="rec")
                nc.vector.reciprocal(rec[:], den[:])
                scl = spool.tile([P128, n_c], F32, tag=f"scale{k}")
                nc.vector.tensor_mul(scl[:], num[:], rec[:])
                nc.vector.tensor_scalar_mul(scl[:], scl[:], SCALE)
                scale_sb.append(scl)

            # ---- per-slot expert MLP ----
            for k in range(S):
                n_c = P[k] // P128
                wg_t, wu_t = [], []
                for h in range(HT):
                    a = wpool.tile([P128, I], F32R, tag="wg", bufs=9, name="wgt")
                    nc.sync.dma_start(out=a[:],
                                      in_=wg_d[k][h * P128:(h + 1) * P128, :])
                    wg_t.append(a)
                    b = wpool.tile([P128, I], F32R, tag="wu", bufs=9, name="wut")
                    nc.sync.dma_start(out=b[:],
                                      in_=wu_d[k][h * P128:(h + 1) * P128, :])
                    wu_t.append(b)
                wd_t = []
                for ii in range(I // P128):
                    d = wdpool.tile([P128, H], F32R, tag="wd", bufs=6, name="wdt")
                    nc.sync.dma_start(out=d[:],
                                      in_=wd_d[k][ii * P128:(ii + 1) * P128, :])
                    wd_t.append(d)

                acts = [apool.tile([P128, P[k]], F32R, tag="acts", bufs=4, name=f"acts{ii}")
                        for ii in range(I // P128)]
                for ii in range(I // P128):
                    for (mo, mw) in _chunks(P[k]):
                        h1 = ps.tile([P128, mw], F32, tag="h1")
                        h2 = ps.tile([P128, mw], F32, tag="h2")
                        for h in range(HT):
                            nc.tensor.matmul(
                                h1[:], lhsT=wg_t[h][:, ii * P128:(ii + 1) * P128],
                                rhs=xe_sb[k][h][:, mo:mo + mw],
                                start=(h == 0), stop=(h == HT - 1))
                        for h in range(HT):
                            nc.tensor.matmul(
                                h2[:], lhsT=wu_t[h][:, ii * P128:(ii + 1) * P128],
                                rhs=xe_sb[k][h][:, mo:mo + mw],
                                start=(h == 0), stop=(h == HT - 1))
                        sl = stpool.tile([P128, mw], F32, tag="silu", bufs=3, name="sl")
                        nc.scalar.activation(sl[:], h1[:], AF.Silu)
                        nc.vector.tensor_mul(acts[ii][:, mo:mo + mw], sl[:], h2[:])

                for cc in range(n_c):
                    for hh in range(2):
                        dps = ps.tile([P128, H // 2], F32, tag="dps")
                        for ii in range(I // P128):
                            nc.tensor.matmul(
                                dps[:],
                                lhsT=acts[ii][:, cc * P128:(cc + 1) * P128],
                                rhs=wd_t[ii][:, hh * (H // 2):(hh + 1) * (H // 2)],
                                start=(ii == 0), stop=(ii == I // P128 - 1))
                        ost = stpool.tile([P128, H // 2], F32, tag="ost")
                        nc.scalar.activation(ost[:], dps[:], AF.Copy,
                                             scale=scale_sb[k][:, cc:cc + 1])
                        nc.sync.dma_start(
                            out=ro_d[k][cc * P128:(cc + 1) * P128,
                                        hh * (H // 2):(hh + 1) * (H // 2)],
                            in_=ost[:])

            # ---- shared expert (intermediate shard ISH=256) ----
            xt_sb = []
            for h in range(HT):
                t = xepool.tile([P128, T], F32R, tag="xt")
                nc.sync.dma_start(out=t[:], in_=xt_d[h * P128:(h + 1) * P128, :])
                xt_sb.append(t)
            wsg_t, wsu_t = [], []
            for h in range(HT):
                a = wpool.tile([P128, ISH], F32R, tag="wsg", bufs=8, name="wsgt")
                nc.sync.dma_start(out=a[:], in_=wsg_d[h * P128:(h + 1) * P128, :])
                wsg_t.append(a)
                b = wpool.tile([P128, ISH], F32R, tag="wsu", bufs=8, name="wsut")
                nc.sync.dma_start(out=b[:], in_=wsu_d[h * P128:(h + 1) * P128, :])
                wsu_t.append(b)
            wsd_t = []
            for ii in range(ISH // P128):
                d = wdpool.tile([P128, H], F32R, tag="wd", bufs=6, name="wsdt")
                nc.sync.dma_start(out=d[:], in_=wsd_d[ii * P128:(ii + 1) * P128, :])
                wsd_t.append(d)

            acts_s = [apool.tile([P128, T], F32R, tag="acts_s", bufs=2, name=f"acts_s{ii}")
                      for ii in range(ISH // P128)]
            for ii in range(ISH // P128):
                for (mo, mw) in _chunks(T):
                    h1 = ps.tile([P128, mw], F32, tag="h1")
                    h2 = ps.tile([P128, mw], F32, tag="h2")
                    for h in range(HT):
                        nc.tensor.matmul(
                            h1[:], lhsT=wsg_t[h][:, ii * P128:(ii + 1) * P128],
                            rhs=xt_sb[h][:, mo:mo + mw],
                            start=(h == 0), stop=(h == HT - 1))
                    for h in range(HT):
                        nc.tensor.matmul(
                            h2[:], lhsT=wsu_t[h][:, ii * P128:(ii + 1) * P128],
                            rhs=xt_sb[h][:, mo:mo + mw],
                            start=(h == 0), stop=(h == HT - 1))
                    sl = stpool.tile([P128, mw], F32, tag="silu", bufs=3, name="sl")
                    nc.scalar.activation(sl[:], h1[:], AF.Silu)
                    nc.vector.tensor_mul(acts_s[ii][:, mo:mo + mw], sl[:], h2[:])

            for cc in range(T // P128):
                for hh in range(2):
                    dps = ps.tile([P128, H // 2], F32, tag="dps")
                    for ii in range(ISH // P128):
                        nc.tensor.matmul(
                            dps[:],
                            lhsT=acts_s[ii][:, cc * P128:(cc + 1) * P128],
                            rhs=wsd_t[ii][:, hh * (H // 2):(hh + 1) * (H // 2)],
                            start=(ii == 0), stop=(ii == ISH // P128 - 1))
                    ost = stpool.tile([P128, H // 2], F32, tag="ost")
                    nc.vector.tensor_copy(ost[:], dps[:])
                    nc.sync.dma_start(
                        out=so_d[cc * P128:(cc + 1) * P128,
                                 hh * (H // 2):(hh + 1) * (H // 2)],
                        in_=ost[:])

    nc.compile()
    return nc


def _prepare(inputs):
    """Host-side dispatch prep: returns (in_maps, P, slot_expert, tok_lists)."""
    x = np.ascontiguousarray(inputs["hidden_states"], dtype=np.float32)
    gate_w = np.asarray(inputs["gate_w"], dtype=np.float32)
    e_bias = np.asarray(inputs["e_bias"], dtype=np.float32)
    w_gate = np.asarray(inputs["w_gate"], dtype=np.float32)
    w_up = np.asarray(inputs["w_up"], dtype=np.float32)
    w_down = np.asarray(inputs["w_down"], dtype=np.float32)
    ws_gate = np.asarray(inputs["ws_gate"], dtype=np.float32)
    ws_up = np.asarray(inputs["ws_up"], dtype=np.float32)
    ws_down = np.asarray(inputs["ws_down"], dtype=np.float32)

    # ---- dispatch metadata ----
    emask = _host_routing(x, gate_w, e_bias)
    counts = emask.sum(0).astype(np.int64)
    order = np.argsort(-counts, kind="stable")
    tok_lists = [np.nonzero(emask[:, e])[0] for e in range(E)]
    P = [_pad128(max(counts[order[k * NCORES + c]] for c in range(NCORES)))
         for k in range(S)]
    max_nc = P[0] // P128

    xt = np.ascontiguousarray(x.T)
    gwt = np.ascontiguousarray(
        gate_w.T.reshape(H // P128, P128, E).transpose(1, 0, 2))
    in_maps = []
    slot_expert = np.zeros((NCORES, S), dtype=np.int64)
    emf = emask.astype(np.float32)
    for c in range(NCORES):
        m = {"gwt": gwt, "xt": xt,
             "wsg": np.ascontiguousarray(ws_gate[:, c * ISH:(c + 1) * ISH]),
             "wsu": np.ascontiguousarray(ws_up[:, c * ISH:(c + 1) * ISH]),
             "wsd": np.ascontiguousarray(ws_down[c * ISH:(c + 1) * ISH, :])}
        for k in range(S):
            e = int(order[k * NCORES + c])
            slot_expert[c, k] = e
            toks = tok_lists[e]
            xe = np.zeros((H, P[k]), dtype=np.float32)
            xe[:, :len(toks)] = x[toks].T
            n_c = P[k] // P128
            oh = np.zeros((P128, n_c * E), dtype=np.float32)
            oh[:, e::E] = 1.0
            em = np.ones((n_c * P128, E), dtype=np.float32)
            em[:len(toks)] = emf[toks]
            em = np.ascontiguousarray(
                em.reshape(n_c, P128, E).transpose(1, 0, 2).reshape(P128, n_c * E))
            m[f"xe{k}"] = xe
            m[f"wg{k}"] = np.ascontiguousarray(w_gate[e])
            m[f"wu{k}"] = np.ascontiguousarray(w_up[e])
            m[f"wd{k}"] = np.ascontiguousarray(w_down[e])
            m[f"oh{k}"] = oh
            m[f"em{k}"] = em
        in_maps.append(m)

    return in_maps, P, slot_expert, tok_lists


def _recombine(results, slot_expert, tok_lists):
    out = np.zeros((T, H), dtype=np.float32)
    for c in range(NCORES):
        out += results[c]["so"]
    for c in range(NCORES):
        for k in range(S):
            e = slot_expert[c, k]
            toks = tok_lists[e]
            out[toks] += results[c][f"ro{k}"][:len(toks)]
    return out


def kernel(**inputs):
    global LAST_RESULTS
    in_maps, P, slot_expert, tok_lists = _prepare(inputs)
    nc = _build_program(P)
    trace = bool(int(os.environ.get("KERNEL_TRACE", "0")))
    if trace:
        trace = _install_ntff_hook()
    LAST_RESULTS = run_bass_kernel_spmd(
        nc, in_maps, list(range(NCORES)), trace=trace)
    results = LAST_RESULTS.results
    return _recombine(results, slot_expert, tok_lists)
